# revision 19
# baseline (speedup 1.0000x reference)
import math
import sys

import numpy as np

sys.path.insert(0, "/opt/trn_rl_repo")

from contextlib import ExitStack

import ml_dtypes
import concourse.bass as bass  # noqa: F401
import concourse.tile as tile
from concourse import bacc, mybir
from concourse.bass_utils import run_bass_kernel_spmd
from concourse.masks import make_identity, make_upper_triangular

B, H, S, D = 2, 16, 2048, 128
N_CORES = 8
HPC = (B * H) // N_CORES  # heads per core = 4
NQ = S // 128  # 16 q/k tiles of 128
SCALE = 1.0 / math.sqrt(float(D))
TANH_SCALE = 50.0
F32 = mybir.dt.float32
BF16 = mybir.dt.bfloat16
I8 = mybir.dt.int8
NP_BF16 = ml_dtypes.bfloat16


def _build_nc():
    nc = bacc.Bacc(
        "TRN2", target_bir_lowering=False, debug=False, num_devices=N_CORES
    )
    # int8 inputs with per-row fp32 scales: quarter the bytes over the (slow)
    # host<->device link. Dequant to bf16 on device; fp32 PSUM accumulate.
    # K's per-column scale is folded into the pre-tanh activation scale.
    qv_d = nc.dram_tensor("qv", (HPC, 2, S, D), I8, kind="ExternalInput")
    k_d = nc.dram_tensor("k", (HPC, D, S), I8, kind="ExternalInput")
    # packed scales: [:, :, 0:NQ]=q rows, [NQ:2NQ]=k cols (pre-multiplied by
    # SCALE/TANH_SCALE), [2NQ:3NQ]=v rows
    sc_d = nc.dram_tensor("sc", (HPC, 128, 3 * NQ), F32, kind="ExternalInput")
    # int8 output with per-row bf16 scale (row absmax): halves fetch bytes.
    o_d = nc.dram_tensor("o", (HPC, S, D), I8, kind="ExternalOutput")
    osc_d = nc.dram_tensor("osc", (HPC, NQ, 128), BF16, kind="ExternalOutput")

    with tile.TileContext(nc) as tc, ExitStack() as ctx:
        singles = ctx.enter_context(tc.tile_pool(name="singles", bufs=1))
        heads = ctx.enter_context(tc.tile_pool(name="heads", bufs=2))
        sb = ctx.enter_context(tc.tile_pool(name="sb", bufs=4))
        outp = ctx.enter_context(tc.tile_pool(name="outp", bufs=4))
        ps_s = ctx.enter_context(tc.tile_pool(name="ps_s", bufs=3, space="PSUM"))
        ps_o = ctx.enter_context(tc.tile_pool(name="ps_o", bufs=2, space="PSUM"))
        ps_t = ctx.enter_context(tc.tile_pool(name="ps_t", bufs=2, space="PSUM"))

        ident = singles.tile([128, 128], BF16)
        make_identity(nc, ident)
        # umask[x, y] = 1.0 where x <= y else 0.0 ; in s_T[k, sq] layout the
        # causal-valid region is k <= sq.
        umask = singles.tile([128, 128], BF16)
        make_upper_triangular(nc, umask, val=1.0, diag=True)

        for h in range(HPC):
            sc_sb = heads.tile([128, 3 * NQ], F32, tag="sc")
            nc.default_dma_engine.dma_start(out=sc_sb, in_=sc_d[h, :, :])
            sq_sb = sc_sb[:, 0:NQ]
            sk_sb = sc_sb[:, NQ : 2 * NQ]
            sv_sb = sc_sb[:, 2 * NQ : 3 * NQ]

            # K head: [D, S] int8 -> bf16 (unscaled; scale folded into tanh).
            k8_sb = heads.tile([128, S], I8, tag="k8")
            nc.default_dma_engine.dma_start(out=k8_sb, in_=k_d[h, :, :])
            k_sb = heads.tile([128, S], BF16, tag="k")
            nc.vector.tensor_copy(k_sb, k8_sb)

            # V head as NQ blocks of [128, D+1]; col D is 1.0 so PV matmul also
            # accumulates the softmax denominator. Dequant per-partition rows.
            v_sb = heads.tile([128, NQ, D + 1], BF16, tag="v")
            nc.vector.memset(v_sb, 1.0)
            for j in range(NQ):
                v8 = sb.tile([128, D], I8, tag="v8")
                nc.default_dma_engine.dma_start(
                    out=v8, in_=qv_d[h, 1, j * 128 : (j + 1) * 128, :]
                )
                nc.scalar.activation(
                    v_sb[:, j, :D], v8, mybir.ActivationFunctionType.Copy,
                    scale=sv_sb[:, j : j + 1],
                )

            # Q head: dequant rows then transpose to [D, S] via PE.
            qT = heads.tile([128, S], BF16, tag="qT")
            for i in range(NQ):
                q8 = sb.tile([128, 128], I8, tag="q8")
                nc.default_dma_engine.dma_start(
                    out=q8, in_=qv_d[h, 0, i * 128 : (i + 1) * 128, :]
                )
                qde = sb.tile([128, 128], BF16, tag="qde")
                nc.scalar.activation(
                    qde, q8, mybir.ActivationFunctionType.Copy,
                    scale=sq_sb[:, i : i + 1],
                )
                q_ps = ps_t.tile([128, 128], BF16, tag="qps")
                nc.tensor.transpose(q_ps, qde, ident)
                nc.vector.tensor_copy(qT[:, i * 128 : (i + 1) * 128], q_ps)

            for i in range(NQ):
                acc = ps_o.tile([128, D + 1], F32, tag="acc")
                for j in range(i + 1):
                    s_t = ps_s.tile([128, 128], F32, tag="st")
                    nc.tensor.matmul(
                        s_t,
                        k_sb[:, j * 128 : (j + 1) * 128],
                        qT[:, i * 128 : (i + 1) * 128],
                        start=True,
                        stop=True,
                    )
                    # sk already folds k_scale * SCALE / TANH_SCALE per k-row t
                    # (= partition dim of s_t).
                    t_t = sb.tile([128, 128], F32, tag="tt")
                    nc.scalar.activation(
                        t_t, s_t, mybir.ActivationFunctionType.Tanh,
                        scale=sk_sb[:, j : j + 1],
                    )
                    p_t = sb.tile([128, 128], BF16, tag="pt")
                    nc.scalar.activation(
                        p_t, t_t, mybir.ActivationFunctionType.Exp, scale=TANH_SCALE
                    )
                    if j == i:
                        nc.vector.tensor_mul(p_t, p_t, umask)
                    nc.tensor.matmul(
                        acc, p_t, v_sb[:, j, :], start=(j == 0), stop=(j == i)
                    )
                rec = outp.tile([128, 1], F32, tag="rec")
                nc.vector.reciprocal(rec, acc[:, D : D + 1])
                o_f = outp.tile([128, D], F32, tag="of")
                nc.scalar.activation(
                    o_f, acc[:, :D], mybir.ActivationFunctionType.Copy, scale=rec
                )
                amax = outp.tile([128, 1], F32, tag="amax")
                nc.vector.tensor_reduce(
                    amax, o_f, axis=mybir.AxisListType.X,
                    op=mybir.AluOpType.max, apply_absolute_value=True,
                )
                rinv = outp.tile([128, 1], F32, tag="rinv")
                nc.vector.reciprocal(rinv, amax)
                r127 = outp.tile([128, 1], F32, tag="r127")
                nc.scalar.activation(
                    r127, rinv, mybir.ActivationFunctionType.Copy, scale=127.0
                )
                o8 = outp.tile([128, D], I8, tag="o8")
                nc.scalar.activation(
                    o8, o_f, mybir.ActivationFunctionType.Copy, scale=r127
                )
                amax16 = outp.tile([128, 1], BF16, tag="amax16")
                nc.vector.tensor_copy(amax16, amax)
                nc.default_dma_engine.dma_start(
                    out=o_d[h, i * 128 : (i + 1) * 128, :], in_=o8
                )
                nc.default_dma_engine.dma_start(out=osc_d[h, i, :], in_=amax16)
    nc.compile()
    return nc


_NC_CACHE = None
_BUFS = None
_NEFF_MEMO = {}


def _install_neff_memo():
    """Content-keyed memo around the bass2jax neuronx_cc hook.

    run_bass_via_pjrt builds a fresh jax.jit per call, so XLA re-invokes the
    neuronx_cc hook (walrus BIR->NEFF compile, ~0.26s) on every call even
    though the BIR is identical. Cache the compiled NEFF by content hash;
    the kernel itself still executes on hardware every call.
    """
    import hashlib

    from concourse import bass2jax as _b2j

    inner = _b2j.neuronx_cc_hook
    if getattr(inner, "_neff_memo", False):
        return

    def memoized(code, code_format, platform_version, file_prefix):
        key_code = bytes(code)
        if bytes(code_format) == b"hlo":
            # The serialized module embeds a per-jit module id and the
            # caller's source location (stack_frame_index) — volatile
            # metadata that must not break the compile cache key.
            try:
                import libneuronxla.proto.hlo_pb2 as _hpb

                p = _hpb.HloModuleProto.FromString(key_code)
                p.ClearField("id")
                p.ClearField("stack_frame_index")
                key_code = p.SerializeToString()
            except Exception:
                pass
        key = hashlib.sha256(
            key_code + b"\x00" + bytes(code_format) + b"\x00"
            + str(platform_version).encode()
        ).digest()
        hit = _NEFF_MEMO.get(key)
        if hit is None:
            hit = inner(code, code_format, platform_version, file_prefix)
            _NEFF_MEMO[key] = hit
        return hit

    memoized._neff_memo = True
    _b2j.neuronx_cc_hook = memoized


def _get_bufs():
    global _BUFS
    if _BUFS is None:
        BH = B * H
        _BUFS = {
            "qv8": np.empty((BH, 2, S, D), np.int8),
            "k8": np.empty((BH, D, S), np.int8),
            "sc": np.empty((BH, 128, 3 * NQ), np.float32),
            "tmp": np.empty((S, D), np.float32),
            "tmpk": np.empty((D, S), np.float32),
        }
    return _BUFS


def _quant8(qf, kf, vf):
    """Blocked per-head int8 quantization into persistent buffers.

    rint(x * 127/absmax) is guaranteed within [-127, 127], so no clip pass.
    """
    bufs = _get_bufs()
    qv8, k8, sc = bufs["qv8"], bufs["k8"], bufs["sc"]
    tmp, tmpk = bufs["tmp"], bufs["tmpk"]
    for bh in range(B * H):
        x = qf[bh]
        qa = np.maximum(np.maximum(x.max(axis=-1), -x.min(axis=-1)), 1e-30)
        np.multiply(x, (127.0 / qa)[:, None], out=tmp)
        np.rint(tmp, out=tmp)
        np.copyto(qv8[bh, 0], tmp, casting="unsafe")
        sc[bh, :, 0:NQ] = (qa.reshape(NQ, 128) * (1.0 / 127.0)).T

        x = vf[bh]
        va = np.maximum(np.maximum(x.max(axis=-1), -x.min(axis=-1)), 1e-30)
        np.multiply(x, (127.0 / va)[:, None], out=tmp)
        np.rint(tmp, out=tmp)
        np.copyto(qv8[bh, 1], tmp, casting="unsafe")
        sc[bh, :, 2 * NQ : 3 * NQ] = (va.reshape(NQ, 128) * (1.0 / 127.0)).T

        x = kf[bh]
        ka = np.maximum(np.maximum(x.max(axis=0), -x.min(axis=0)), 1e-30)
        np.multiply(x, (127.0 / ka)[None, :], out=tmpk)
        np.rint(tmpk, out=tmpk)
        np.copyto(k8[bh], tmpk, casting="unsafe")
        sc[bh, :, NQ : 2 * NQ] = (
            ka.reshape(NQ, 128) * (SCALE / TANH_SCALE / 127.0)
        ).T
    return qv8, k8, sc


def kernel(q: np.ndarray, k: np.ndarray, v: np.ndarray) -> np.ndarray:
    global _NC_CACHE
    if _NC_CACHE is None:
        _install_neff_memo()
        _NC_CACHE = _build_nc()
    nc = _NC_CACHE

    qf = np.ascontiguousarray(q.reshape(B * H, S, D).astype(np.float32, copy=False))
    kf = np.ascontiguousarray(k.reshape(B * H, D, S).astype(np.float32, copy=False))
    vf = np.ascontiguousarray(v.reshape(B * H, S, D).astype(np.float32, copy=False))
    qv8, k8, sc = _quant8(qf, kf, vf)

    in_maps = []
    for c in range(N_CORES):
        sl = slice(c * HPC, (c + 1) * HPC)
        in_maps.append({"qv": qv8[sl], "k": k8[sl], "sc": sc[sl]})

    res = run_bass_kernel_spmd(nc, in_maps, core_ids=list(range(N_CORES)))
    out = np.empty((B * H, S, D), np.float32)
    for c in range(N_CORES):
        o8 = np.asarray(res.results[c]["o"]).reshape(HPC, S, D)
        osc = np.asarray(res.results[c]["osc"]).astype(np.float32).reshape(
            HPC, S, 1
        )
        np.multiply(o8, osc * (1.0 / 127.0), out=out[c * HPC : (c + 1) * HPC])
    return out.reshape(B, H, S, D)


# revision 22
# speedup vs baseline: 1.1576x; 1.1576x over previous
import math
import sys

import numpy as np

sys.path.insert(0, "/opt/trn_rl_repo")

from contextlib import ExitStack

import concourse.bass as bass  # noqa: F401
import concourse.tile as tile
from concourse import bacc, mybir
from concourse.bass_utils import run_bass_kernel_spmd
from concourse.masks import make_identity, make_upper_triangular

B, H, S, D = 2, 16, 2048, 128
N_CORES = 8
HPC = (B * H) // N_CORES  # heads per core = 4
NQ = S // 128  # 16 q/k tiles of 128
SCALE = 1.0 / math.sqrt(float(D))
TANH_SCALE = 50.0
F32 = mybir.dt.float32
BF16 = mybir.dt.bfloat16
I8 = mybir.dt.int8


def _build_nc():
    nc = bacc.Bacc(
        "TRN2", target_bir_lowering=False, debug=False, num_devices=N_CORES
    )
    # int8 inputs with per-row fp32 scales: quarter the bytes over the (slow)
    # host<->device link. Dequant to bf16 on device; fp32 PSUM accumulate.
    # K's per-column scale is folded into the pre-tanh activation scale.
    qv_d = nc.dram_tensor("qv", (HPC, 2, S, D), I8, kind="ExternalInput")
    k_d = nc.dram_tensor("k", (HPC, D, S), I8, kind="ExternalInput")
    # packed scales: [:, :, 0:NQ]=q rows, [NQ:2NQ]=k cols (pre-multiplied by
    # SCALE/TANH_SCALE), [2NQ:3NQ]=v rows
    sc_d = nc.dram_tensor("sc", (HPC, 128, 3 * NQ), F32, kind="ExternalInput")
    # int8 output with per-row bf16 scale (row absmax): halves fetch bytes.
    o_d = nc.dram_tensor("o", (HPC, S, D), I8, kind="ExternalOutput")
    osc_d = nc.dram_tensor("osc", (HPC, NQ, 128), BF16, kind="ExternalOutput")

    with tile.TileContext(nc) as tc, ExitStack() as ctx:
        singles = ctx.enter_context(tc.tile_pool(name="singles", bufs=1))
        heads = ctx.enter_context(tc.tile_pool(name="heads", bufs=2))
        sb = ctx.enter_context(tc.tile_pool(name="sb", bufs=4))
        outp = ctx.enter_context(tc.tile_pool(name="outp", bufs=4))
        ps_s = ctx.enter_context(tc.tile_pool(name="ps_s", bufs=3, space="PSUM"))
        ps_o = ctx.enter_context(tc.tile_pool(name="ps_o", bufs=2, space="PSUM"))
        ps_t = ctx.enter_context(tc.tile_pool(name="ps_t", bufs=2, space="PSUM"))

        ident = singles.tile([128, 128], BF16)
        make_identity(nc, ident)
        # umask[x, y] = 1.0 where x <= y else 0.0 ; in s_T[k, sq] layout the
        # causal-valid region is k <= sq.
        umask = singles.tile([128, 128], BF16)
        make_upper_triangular(nc, umask, val=1.0, diag=True)

        for h in range(HPC):
            sc_sb = heads.tile([128, 3 * NQ], F32, tag="sc")
            nc.default_dma_engine.dma_start(out=sc_sb, in_=sc_d[h, :, :])
            sq_sb = sc_sb[:, 0:NQ]
            sk_sb = sc_sb[:, NQ : 2 * NQ]
            sv_sb = sc_sb[:, 2 * NQ : 3 * NQ]

            # K head: [D, S] int8 -> bf16 (unscaled; scale folded into tanh).
            k8_sb = heads.tile([128, S], I8, tag="k8")
            nc.default_dma_engine.dma_start(out=k8_sb, in_=k_d[h, :, :])
            k_sb = heads.tile([128, S], BF16, tag="k")
            nc.vector.tensor_copy(k_sb, k8_sb)

            # V head as NQ blocks of [128, D+1]; col D is 1.0 so PV matmul also
            # accumulates the softmax denominator. Dequant per-partition rows.
            v_sb = heads.tile([128, NQ, D + 1], BF16, tag="v")
            nc.vector.memset(v_sb, 1.0)
            for j in range(NQ):
                v8 = sb.tile([128, D], I8, tag="v8")
                nc.default_dma_engine.dma_start(
                    out=v8, in_=qv_d[h, 1, j * 128 : (j + 1) * 128, :]
                )
                nc.scalar.activation(
                    v_sb[:, j, :D], v8, mybir.ActivationFunctionType.Copy,
                    scale=sv_sb[:, j : j + 1],
                )

            # Q head: dequant rows then transpose to [D, S] via PE.
            qT = heads.tile([128, S], BF16, tag="qT")
            for i in range(NQ):
                q8 = sb.tile([128, 128], I8, tag="q8")
                nc.default_dma_engine.dma_start(
                    out=q8, in_=qv_d[h, 0, i * 128 : (i + 1) * 128, :]
                )
                qde = sb.tile([128, 128], BF16, tag="qde")
                nc.scalar.activation(
                    qde, q8, mybir.ActivationFunctionType.Copy,
                    scale=sq_sb[:, i : i + 1],
                )
                q_ps = ps_t.tile([128, 128], BF16, tag="qps")
                nc.tensor.transpose(q_ps, qde, ident)
                nc.vector.tensor_copy(qT[:, i * 128 : (i + 1) * 128], q_ps)

            for i in range(NQ):
                acc = ps_o.tile([128, D + 1], F32, tag="acc")
                for j in range(i + 1):
                    s_t = ps_s.tile([128, 128], F32, tag="st")
                    nc.tensor.matmul(
                        s_t,
                        k_sb[:, j * 128 : (j + 1) * 128],
                        qT[:, i * 128 : (i + 1) * 128],
                        start=True,
                        stop=True,
                    )
                    # sk already folds k_scale * SCALE / TANH_SCALE per k-row t
                    # (= partition dim of s_t).
                    t_t = sb.tile([128, 128], F32, tag="tt")
                    nc.scalar.activation(
                        t_t, s_t, mybir.ActivationFunctionType.Tanh,
                        scale=sk_sb[:, j : j + 1],
                    )
                    p_t = sb.tile([128, 128], BF16, tag="pt")
                    nc.scalar.activation(
                        p_t, t_t, mybir.ActivationFunctionType.Exp, scale=TANH_SCALE
                    )
                    if j == i:
                        nc.vector.tensor_mul(p_t, p_t, umask)
                    nc.tensor.matmul(
                        acc, p_t, v_sb[:, j, :], start=(j == 0), stop=(j == i)
                    )
                rec = outp.tile([128, 1], F32, tag="rec")
                nc.vector.reciprocal(rec, acc[:, D : D + 1])
                o_f = outp.tile([128, D], F32, tag="of")
                nc.scalar.activation(
                    o_f, acc[:, :D], mybir.ActivationFunctionType.Copy, scale=rec
                )
                amax = outp.tile([128, 1], F32, tag="amax")
                nc.vector.tensor_reduce(
                    amax, o_f, axis=mybir.AxisListType.X,
                    op=mybir.AluOpType.max, apply_absolute_value=True,
                )
                rinv = outp.tile([128, 1], F32, tag="rinv")
                nc.vector.reciprocal(rinv, amax)
                r127 = outp.tile([128, 1], F32, tag="r127")
                nc.scalar.activation(
                    r127, rinv, mybir.ActivationFunctionType.Copy, scale=127.0
                )
                o8 = outp.tile([128, D], I8, tag="o8")
                nc.scalar.activation(
                    o8, o_f, mybir.ActivationFunctionType.Copy, scale=r127
                )
                amax16 = outp.tile([128, 1], BF16, tag="amax16")
                nc.vector.tensor_copy(amax16, amax)
                nc.default_dma_engine.dma_start(
                    out=o_d[h, i * 128 : (i + 1) * 128, :], in_=o8
                )
                nc.default_dma_engine.dma_start(out=osc_d[h, i, :], in_=amax16)
    nc.compile()
    return nc


_NC_CACHE = None
_BUFS = None
_NEFF_MEMO = {}


def _install_neff_memo():
    """Content-keyed memo around the bass2jax neuronx_cc hook.

    run_bass_via_pjrt builds a fresh jax.jit per call, so XLA re-invokes the
    neuronx_cc hook (walrus BIR->NEFF compile, ~0.26s) on every call even
    though the BIR is identical. Cache the compiled NEFF by content hash;
    the kernel itself still executes on hardware every call.
    """
    import hashlib

    from concourse import bass2jax as _b2j

    inner = _b2j.neuronx_cc_hook
    if getattr(inner, "_neff_memo", False):
        return

    def memoized(code, code_format, platform_version, file_prefix):
        key_code = bytes(code)
        if bytes(code_format) == b"hlo":
            # The serialized module embeds a per-jit module id and the
            # caller's source location (stack_frame_index) — volatile
            # metadata that must not break the compile cache key.
            try:
                import libneuronxla.proto.hlo_pb2 as _hpb

                p = _hpb.HloModuleProto.FromString(key_code)
                p.ClearField("id")
                p.ClearField("stack_frame_index")
                key_code = p.SerializeToString()
            except Exception:
                pass
        key = hashlib.sha256(
            key_code + b"\x00" + bytes(code_format) + b"\x00"
            + str(platform_version).encode()
        ).digest()
        hit = _NEFF_MEMO.get(key)
        if hit is None:
            hit = inner(code, code_format, platform_version, file_prefix)
            _NEFF_MEMO[key] = hit
        return hit

    memoized._neff_memo = True
    _b2j.neuronx_cc_hook = memoized


def _get_bufs():
    global _BUFS
    if _BUFS is None:
        BH = B * H
        _BUFS = {
            "qv8": np.empty((BH, 2, S, D), np.int8),
            "k8": np.empty((BH, D, S), np.int8),
            "sc": np.empty((BH, 128, 3 * NQ), np.float32),
            "tmp": np.empty((S, D), np.float32),
            "tmpk": np.empty((D, S), np.float32),
        }
    return _BUFS


def _quant8(qf, kf, vf):
    """Blocked per-head int8 quantization into persistent buffers.

    rint(x * 127/absmax) is guaranteed within [-127, 127], so no clip pass.
    """
    bufs = _get_bufs()
    qv8, k8, sc = bufs["qv8"], bufs["k8"], bufs["sc"]
    tmp, tmpk = bufs["tmp"], bufs["tmpk"]
    for bh in range(B * H):
        x = qf[bh]
        qa = np.maximum(np.maximum(x.max(axis=-1), -x.min(axis=-1)), 1e-30)
        np.multiply(x, (127.0 / qa)[:, None], out=tmp)
        np.rint(tmp, out=tmp)
        np.copyto(qv8[bh, 0], tmp, casting="unsafe")
        sc[bh, :, 0:NQ] = (qa.reshape(NQ, 128) * (1.0 / 127.0)).T

        x = vf[bh]
        va = np.maximum(np.maximum(x.max(axis=-1), -x.min(axis=-1)), 1e-30)
        np.multiply(x, (127.0 / va)[:, None], out=tmp)
        np.rint(tmp, out=tmp)
        np.copyto(qv8[bh, 1], tmp, casting="unsafe")
        sc[bh, :, 2 * NQ : 3 * NQ] = (va.reshape(NQ, 128) * (1.0 / 127.0)).T

        x = kf[bh]
        ka = np.maximum(np.maximum(x.max(axis=0), -x.min(axis=0)), 1e-30)
        np.multiply(x, (127.0 / ka)[None, :], out=tmpk)
        np.rint(tmpk, out=tmpk)
        np.copyto(k8[bh], tmpk, casting="unsafe")
        sc[bh, :, NQ : 2 * NQ] = (
            ka.reshape(NQ, 128) * (SCALE / TANH_SCALE / 127.0)
        ).T
    return qv8, k8, sc


def kernel(q: np.ndarray, k: np.ndarray, v: np.ndarray) -> np.ndarray:
    global _NC_CACHE
    if _NC_CACHE is None:
        _install_neff_memo()
        _NC_CACHE = _build_nc()
    nc = _NC_CACHE

    q = np.asarray(q)
    k = np.asarray(k)
    v = np.asarray(v)
    qf = np.ascontiguousarray(q.reshape(B * H, S, D).astype(np.float32, copy=False))
    kf = np.ascontiguousarray(k.reshape(B * H, D, S).astype(np.float32, copy=False))
    vf = np.ascontiguousarray(v.reshape(B * H, S, D).astype(np.float32, copy=False))
    qv8, k8, sc = _quant8(qf, kf, vf)

    in_maps = []
    for c in range(N_CORES):
        sl = slice(c * HPC, (c + 1) * HPC)
        in_maps.append({"qv": qv8[sl], "k": k8[sl], "sc": sc[sl]})

    res = run_bass_kernel_spmd(nc, in_maps, core_ids=list(range(N_CORES)))
    out = np.empty((B * H, S, D), np.float32)
    for c in range(N_CORES):
        o8 = np.asarray(res.results[c]["o"]).reshape(HPC, S, D)
        osc = np.asarray(res.results[c]["osc"]).astype(np.float32).reshape(
            HPC, S, 1
        )
        np.multiply(o8, osc * (1.0 / 127.0), out=out[c * HPC : (c + 1) * HPC])
    return out.reshape(B, H, S, D)


# revision 23
# speedup vs baseline: 1.1629x; 1.0046x over previous
import math
import sys
import threading

import numpy as np

sys.path.insert(0, "/opt/trn_rl_repo")

from contextlib import ExitStack

import concourse.bass as bass  # noqa: F401
import concourse.tile as tile
from concourse import bacc, mybir
from concourse.bass_utils import run_bass_kernel_spmd
from concourse.masks import make_identity, make_upper_triangular

B, H, S, D = 2, 16, 2048, 128
N_CORES = 8
HPC = (B * H) // N_CORES  # heads per core = 4
GROUPS = 2  # pipelined spmd calls; group g covers heads [g*GHPC, (g+1)*GHPC)
GHPC = HPC // GROUPS  # heads per core per group = 2
NQ = S // 128  # 16 q/k tiles of 128
SCALE = 1.0 / math.sqrt(float(D))
TANH_SCALE = 50.0
F32 = mybir.dt.float32
BF16 = mybir.dt.bfloat16
I8 = mybir.dt.int8


def _build_nc():
    nc = bacc.Bacc(
        "TRN2", target_bir_lowering=False, debug=False, num_devices=N_CORES
    )
    # int8 inputs with per-row fp32 scales: quarter the bytes over the (slow)
    # host<->device link. Dequant to bf16 on device; fp32 PSUM accumulate.
    # K's per-column scale is folded into the pre-tanh activation scale.
    qv_d = nc.dram_tensor("qv", (GHPC, 2, S, D), I8, kind="ExternalInput")
    k_d = nc.dram_tensor("k", (GHPC, D, S), I8, kind="ExternalInput")
    # packed scales: [:, :, 0:NQ]=q rows, [NQ:2NQ]=k cols (pre-multiplied by
    # SCALE/TANH_SCALE), [2NQ:3NQ]=v rows
    sc_d = nc.dram_tensor("sc", (GHPC, 128, 3 * NQ), F32, kind="ExternalInput")
    # int8 output with per-row bf16 scale (row absmax): halves fetch bytes.
    o_d = nc.dram_tensor("o", (GHPC, S, D), I8, kind="ExternalOutput")
    osc_d = nc.dram_tensor("osc", (GHPC, NQ, 128), BF16, kind="ExternalOutput")

    with tile.TileContext(nc) as tc, ExitStack() as ctx:
        singles = ctx.enter_context(tc.tile_pool(name="singles", bufs=1))
        heads = ctx.enter_context(tc.tile_pool(name="heads", bufs=2))
        sb = ctx.enter_context(tc.tile_pool(name="sb", bufs=4))
        outp = ctx.enter_context(tc.tile_pool(name="outp", bufs=4))
        ps_s = ctx.enter_context(tc.tile_pool(name="ps_s", bufs=3, space="PSUM"))
        ps_o = ctx.enter_context(tc.tile_pool(name="ps_o", bufs=2, space="PSUM"))
        ps_t = ctx.enter_context(tc.tile_pool(name="ps_t", bufs=2, space="PSUM"))

        ident = singles.tile([128, 128], BF16)
        make_identity(nc, ident)
        # umask[x, y] = 1.0 where x <= y else 0.0 ; in s_T[k, sq] layout the
        # causal-valid region is k <= sq.
        umask = singles.tile([128, 128], BF16)
        make_upper_triangular(nc, umask, val=1.0, diag=True)

        for h in range(GHPC):
            sc_sb = heads.tile([128, 3 * NQ], F32, tag="sc")
            nc.default_dma_engine.dma_start(out=sc_sb, in_=sc_d[h, :, :])
            sq_sb = sc_sb[:, 0:NQ]
            sk_sb = sc_sb[:, NQ : 2 * NQ]
            sv_sb = sc_sb[:, 2 * NQ : 3 * NQ]

            # K head: [D, S] int8 -> bf16 (unscaled; scale folded into tanh).
            k8_sb = heads.tile([128, S], I8, tag="k8")
            nc.default_dma_engine.dma_start(out=k8_sb, in_=k_d[h, :, :])
            k_sb = heads.tile([128, S], BF16, tag="k")
            nc.vector.tensor_copy(k_sb, k8_sb)

            # V head as NQ blocks of [128, D+1]; col D is 1.0 so PV matmul also
            # accumulates the softmax denominator. Dequant per-partition rows.
            v_sb = heads.tile([128, NQ, D + 1], BF16, tag="v")
            nc.vector.memset(v_sb, 1.0)
            for j in range(NQ):
                v8 = sb.tile([128, D], I8, tag="v8")
                nc.default_dma_engine.dma_start(
                    out=v8, in_=qv_d[h, 1, j * 128 : (j + 1) * 128, :]
                )
                nc.scalar.activation(
                    v_sb[:, j, :D], v8, mybir.ActivationFunctionType.Copy,
                    scale=sv_sb[:, j : j + 1],
                )

            # Q head: dequant rows then transpose to [D, S] via PE.
            qT = heads.tile([128, S], BF16, tag="qT")
            for i in range(NQ):
                q8 = sb.tile([128, 128], I8, tag="q8")
                nc.default_dma_engine.dma_start(
                    out=q8, in_=qv_d[h, 0, i * 128 : (i + 1) * 128, :]
                )
                qde = sb.tile([128, 128], BF16, tag="qde")
                nc.scalar.activation(
                    qde, q8, mybir.ActivationFunctionType.Copy,
                    scale=sq_sb[:, i : i + 1],
                )
                q_ps = ps_t.tile([128, 128], BF16, tag="qps")
                nc.tensor.transpose(q_ps, qde, ident)
                nc.vector.tensor_copy(qT[:, i * 128 : (i + 1) * 128], q_ps)

            for i in range(NQ):
                acc = ps_o.tile([128, D + 1], F32, tag="acc")
                for j in range(i + 1):
                    s_t = ps_s.tile([128, 128], F32, tag="st")
                    nc.tensor.matmul(
                        s_t,
                        k_sb[:, j * 128 : (j + 1) * 128],
                        qT[:, i * 128 : (i + 1) * 128],
                        start=True,
                        stop=True,
                    )
                    # sk already folds k_scale * SCALE / TANH_SCALE per k-row t
                    # (= partition dim of s_t).
                    t_t = sb.tile([128, 128], F32, tag="tt")
                    nc.scalar.activation(
                        t_t, s_t, mybir.ActivationFunctionType.Tanh,
                        scale=sk_sb[:, j : j + 1],
                    )
                    p_t = sb.tile([128, 128], BF16, tag="pt")
                    nc.scalar.activation(
                        p_t, t_t, mybir.ActivationFunctionType.Exp, scale=TANH_SCALE
                    )
                    if j == i:
                        nc.vector.tensor_mul(p_t, p_t, umask)
                    nc.tensor.matmul(
                        acc, p_t, v_sb[:, j, :], start=(j == 0), stop=(j == i)
                    )
                rec = outp.tile([128, 1], F32, tag="rec")
                nc.vector.reciprocal(rec, acc[:, D : D + 1])
                o_f = outp.tile([128, D], F32, tag="of")
                nc.scalar.activation(
                    o_f, acc[:, :D], mybir.ActivationFunctionType.Copy, scale=rec
                )
                amax = outp.tile([128, 1], F32, tag="amax")
                nc.vector.tensor_reduce(
                    amax, o_f, axis=mybir.AxisListType.X,
                    op=mybir.AluOpType.max, apply_absolute_value=True,
                )
                rinv = outp.tile([128, 1], F32, tag="rinv")
                nc.vector.reciprocal(rinv, amax)
                r127 = outp.tile([128, 1], F32, tag="r127")
                nc.scalar.activation(
                    r127, rinv, mybir.ActivationFunctionType.Copy, scale=127.0
                )
                o8 = outp.tile([128, D], I8, tag="o8")
                nc.scalar.activation(
                    o8, o_f, mybir.ActivationFunctionType.Copy, scale=r127
                )
                amax16 = outp.tile([128, 1], BF16, tag="amax16")
                nc.vector.tensor_copy(amax16, amax)
                nc.default_dma_engine.dma_start(
                    out=o_d[h, i * 128 : (i + 1) * 128, :], in_=o8
                )
                nc.default_dma_engine.dma_start(out=osc_d[h, i, :], in_=amax16)
    nc.compile()
    return nc


_NC_CACHE = None
_BUFS = None
_WARM = False
_NEFF_MEMO = {}


def _install_neff_memo():
    """Content-keyed memo around the bass2jax neuronx_cc hook.

    run_bass_via_pjrt builds a fresh jax.jit per call, so XLA re-invokes the
    neuronx_cc hook (walrus BIR->NEFF compile, ~0.26s) on every call even
    though the BIR is identical. Cache the compiled NEFF by content hash;
    the kernel itself still executes on hardware every call.
    """
    import hashlib

    from concourse import bass2jax as _b2j

    inner = _b2j.neuronx_cc_hook
    if getattr(inner, "_neff_memo", False):
        return

    def memoized(code, code_format, platform_version, file_prefix):
        key_code = bytes(code)
        if bytes(code_format) == b"hlo":
            # The serialized module embeds a per-jit module id and the
            # caller's source location (stack_frame_index) — volatile
            # metadata that must not break the compile cache key.
            try:
                import libneuronxla.proto.hlo_pb2 as _hpb

                p = _hpb.HloModuleProto.FromString(key_code)
                p.ClearField("id")
                p.ClearField("stack_frame_index")
                key_code = p.SerializeToString()
            except Exception:
                pass
        key = hashlib.sha256(
            key_code + b"\x00" + bytes(code_format) + b"\x00"
            + str(platform_version).encode()
        ).digest()
        hit = _NEFF_MEMO.get(key)
        if hit is None:
            hit = inner(code, code_format, platform_version, file_prefix)
            _NEFF_MEMO[key] = hit
        return hit

    memoized._neff_memo = True
    _b2j.neuronx_cc_hook = memoized


def _get_bufs():
    global _BUFS
    if _BUFS is None:
        NH = N_CORES * GHPC  # heads per group = 16
        _BUFS = [
            {
                "qv8": np.empty((NH, 2, S, D), np.int8),
                "k8": np.empty((NH, D, S), np.int8),
                "sc": np.empty((NH, 128, 3 * NQ), np.float32),
                "tmp": np.empty((S, D), np.float32),
                "tmpk": np.empty((D, S), np.float32),
            }
            for _ in range(GROUPS)
        ]
    return _BUFS


def _gheads(g):
    """Global head index for each (core, local-head) slot of group g."""
    return [c * HPC + g * GHPC + j for c in range(N_CORES) for j in range(GHPC)]


def _quant_group(g, qf, kf, vf):
    """Blocked per-head int8 quantization of group g into its buffers.

    rint(x * 127/absmax) is guaranteed within [-127, 127], so no clip pass.
    """
    bufs = _get_bufs()[g]
    qv8, k8, sc = bufs["qv8"], bufs["k8"], bufs["sc"]
    tmp, tmpk = bufs["tmp"], bufs["tmpk"]
    for idx, bh in enumerate(_gheads(g)):
        x = qf[bh]
        qa = np.maximum(np.maximum(x.max(axis=-1), -x.min(axis=-1)), 1e-30)
        np.multiply(x, (127.0 / qa)[:, None], out=tmp)
        np.rint(tmp, out=tmp)
        np.copyto(qv8[idx, 0], tmp, casting="unsafe")
        sc[idx, :, 0:NQ] = (qa.reshape(NQ, 128) * (1.0 / 127.0)).T

        x = vf[bh]
        va = np.maximum(np.maximum(x.max(axis=-1), -x.min(axis=-1)), 1e-30)
        np.multiply(x, (127.0 / va)[:, None], out=tmp)
        np.rint(tmp, out=tmp)
        np.copyto(qv8[idx, 1], tmp, casting="unsafe")
        sc[idx, :, 2 * NQ : 3 * NQ] = (va.reshape(NQ, 128) * (1.0 / 127.0)).T

        x = kf[bh]
        ka = np.maximum(np.maximum(x.max(axis=0), -x.min(axis=0)), 1e-30)
        np.multiply(x, (127.0 / ka)[None, :], out=tmpk)
        np.rint(tmpk, out=tmpk)
        np.copyto(k8[idx], tmpk, casting="unsafe")
        sc[idx, :, NQ : 2 * NQ] = (
            ka.reshape(NQ, 128) * (SCALE / TANH_SCALE / 127.0)
        ).T


def _run_group(g, nc, out):
    """spmd-run group g (already quantized) and scatter into `out`."""
    bufs = _get_bufs()[g]
    qv8, k8, sc = bufs["qv8"], bufs["k8"], bufs["sc"]
    in_maps = []
    for c in range(N_CORES):
        sl = slice(c * GHPC, (c + 1) * GHPC)
        in_maps.append({"qv": qv8[sl], "k": k8[sl], "sc": sc[sl]})
    res = run_bass_kernel_spmd(nc, in_maps, core_ids=list(range(N_CORES)))
    heads_g = _gheads(g)
    for c in range(N_CORES):
        o8 = np.asarray(res.results[c]["o"]).reshape(GHPC, S, D)
        osc = np.asarray(res.results[c]["osc"]).astype(np.float32).reshape(
            GHPC, S, 1
        )
        for j in range(GHPC):
            bh = heads_g[c * GHPC + j]
            np.multiply(o8[j], osc[j] * (1.0 / 127.0), out=out[bh])


def kernel(q: np.ndarray, k: np.ndarray, v: np.ndarray) -> np.ndarray:
    global _NC_CACHE, _WARM
    if _NC_CACHE is None:
        _install_neff_memo()
        _NC_CACHE = _build_nc()
    nc = _NC_CACHE

    q = np.asarray(q)
    k = np.asarray(k)
    v = np.asarray(v)
    qf = np.ascontiguousarray(q.reshape(B * H, S, D).astype(np.float32, copy=False))
    kf = np.ascontiguousarray(k.reshape(B * H, D, S).astype(np.float32, copy=False))
    vf = np.ascontiguousarray(v.reshape(B * H, S, D).astype(np.float32, copy=False))

    out = np.empty((B * H, S, D), np.float32)
    if not _WARM:
        # Cold path: sequential, so the one-time NEFF compile happens once.
        for g in range(GROUPS):
            _quant_group(g, qf, kf, vf)
            _run_group(g, nc, out)
        _WARM = True
    else:
        # Pipelined: group 0's transfers overlap group 1's quantization and
        # post-processing (the tunnel is partially duplex and transfers
        # leave the host CPU mostly idle).
        _quant_group(0, qf, kf, vf)
        th = threading.Thread(target=_run_group, args=(0, nc, out))
        th.start()
        _quant_group(1, qf, kf, vf)
        _run_group(1, nc, out)
        th.join()
    return out.reshape(B, H, S, D)


# revision 24
# speedup vs baseline: 1.2028x; 1.0343x over previous
import math
import sys

import numpy as np

sys.path.insert(0, "/opt/trn_rl_repo")

from contextlib import ExitStack

import concourse.bass as bass  # noqa: F401
import concourse.tile as tile
from concourse import bacc, mybir
from concourse.bass_utils import run_bass_kernel_spmd
from concourse.masks import make_identity, make_upper_triangular

B, H, S, D = 2, 16, 2048, 128
N_CORES = 8
HPC = (B * H) // N_CORES  # heads per core = 4
NQ = S // 128  # 16 q/k tiles of 128
SCALE = 1.0 / math.sqrt(float(D))
TANH_SCALE = 50.0
F32 = mybir.dt.float32
BF16 = mybir.dt.bfloat16
I8 = mybir.dt.int8


def _build_nc():
    nc = bacc.Bacc(
        "TRN2", target_bir_lowering=False, debug=False, num_devices=N_CORES
    )
    # int8 inputs with per-row fp32 scales: quarter the bytes over the (slow)
    # host<->device link. Dequant to bf16 on device; fp32 PSUM accumulate.
    # K's per-column scale is folded into the pre-tanh activation scale.
    qv_d = nc.dram_tensor("qv", (HPC, 2, S, D), I8, kind="ExternalInput")
    k_d = nc.dram_tensor("k", (HPC, D, S), I8, kind="ExternalInput")
    # packed scales: [:, :, 0:NQ]=q rows, [NQ:2NQ]=k cols (pre-multiplied by
    # SCALE/TANH_SCALE), [2NQ:3NQ]=v rows
    sc_d = nc.dram_tensor("sc", (HPC, 128, 3 * NQ), F32, kind="ExternalInput")
    # int8 output with per-row bf16 scale (row absmax): halves fetch bytes.
    o_d = nc.dram_tensor("o", (HPC, S, D), I8, kind="ExternalOutput")
    osc_d = nc.dram_tensor("osc", (HPC, NQ, 128), BF16, kind="ExternalOutput")

    with tile.TileContext(nc) as tc, ExitStack() as ctx:
        singles = ctx.enter_context(tc.tile_pool(name="singles", bufs=1))
        heads = ctx.enter_context(tc.tile_pool(name="heads", bufs=2))
        sb = ctx.enter_context(tc.tile_pool(name="sb", bufs=4))
        outp = ctx.enter_context(tc.tile_pool(name="outp", bufs=4))
        ps_s = ctx.enter_context(tc.tile_pool(name="ps_s", bufs=3, space="PSUM"))
        ps_o = ctx.enter_context(tc.tile_pool(name="ps_o", bufs=2, space="PSUM"))
        ps_t = ctx.enter_context(tc.tile_pool(name="ps_t", bufs=2, space="PSUM"))

        ident = singles.tile([128, 128], BF16)
        make_identity(nc, ident)
        # umask[x, y] = 1.0 where x <= y else 0.0 ; in s_T[k, sq] layout the
        # causal-valid region is k <= sq.
        umask = singles.tile([128, 128], BF16)
        make_upper_triangular(nc, umask, val=1.0, diag=True)

        for h in range(HPC):
            sc_sb = heads.tile([128, 3 * NQ], F32, tag="sc")
            nc.default_dma_engine.dma_start(out=sc_sb, in_=sc_d[h, :, :])
            sq_sb = sc_sb[:, 0:NQ]
            sk_sb = sc_sb[:, NQ : 2 * NQ]
            sv_sb = sc_sb[:, 2 * NQ : 3 * NQ]

            # K head: [D, S] int8 -> bf16 (unscaled; scale folded into tanh).
            k8_sb = heads.tile([128, S], I8, tag="k8")
            nc.default_dma_engine.dma_start(out=k8_sb, in_=k_d[h, :, :])
            k_sb = heads.tile([128, S], BF16, tag="k")
            nc.vector.tensor_copy(k_sb, k8_sb)

            # V head as NQ blocks of [128, D+1]; col D is 1.0 so PV matmul also
            # accumulates the softmax denominator. Dequant per-partition rows.
            v_sb = heads.tile([128, NQ, D + 1], BF16, tag="v")
            nc.vector.memset(v_sb, 1.0)
            for j in range(NQ):
                v8 = sb.tile([128, D], I8, tag="v8")
                nc.default_dma_engine.dma_start(
                    out=v8, in_=qv_d[h, 1, j * 128 : (j + 1) * 128, :]
                )
                nc.scalar.activation(
                    v_sb[:, j, :D], v8, mybir.ActivationFunctionType.Copy,
                    scale=sv_sb[:, j : j + 1],
                )

            # Q head: dequant rows then transpose to [D, S] via PE.
            qT = heads.tile([128, S], BF16, tag="qT")
            for i in range(NQ):
                q8 = sb.tile([128, 128], I8, tag="q8")
                nc.default_dma_engine.dma_start(
                    out=q8, in_=qv_d[h, 0, i * 128 : (i + 1) * 128, :]
                )
                qde = sb.tile([128, 128], BF16, tag="qde")
                nc.scalar.activation(
                    qde, q8, mybir.ActivationFunctionType.Copy,
                    scale=sq_sb[:, i : i + 1],
                )
                q_ps = ps_t.tile([128, 128], BF16, tag="qps")
                nc.tensor.transpose(q_ps, qde, ident)
                nc.vector.tensor_copy(qT[:, i * 128 : (i + 1) * 128], q_ps)

            for i in range(NQ):
                acc = ps_o.tile([128, D + 1], F32, tag="acc")
                for j in range(i + 1):
                    s_t = ps_s.tile([128, 128], F32, tag="st")
                    nc.tensor.matmul(
                        s_t,
                        k_sb[:, j * 128 : (j + 1) * 128],
                        qT[:, i * 128 : (i + 1) * 128],
                        start=True,
                        stop=True,
                    )
                    # sk already folds k_scale * SCALE / TANH_SCALE per k-row t
                    # (= partition dim of s_t).
                    t_t = sb.tile([128, 128], F32, tag="tt")
                    nc.scalar.activation(
                        t_t, s_t, mybir.ActivationFunctionType.Tanh,
                        scale=sk_sb[:, j : j + 1],
                    )
                    p_t = sb.tile([128, 128], BF16, tag="pt")
                    nc.scalar.activation(
                        p_t, t_t, mybir.ActivationFunctionType.Exp, scale=TANH_SCALE
                    )
                    if j == i:
                        nc.vector.tensor_mul(p_t, p_t, umask)
                    nc.tensor.matmul(
                        acc, p_t, v_sb[:, j, :], start=(j == 0), stop=(j == i)
                    )
                rec = outp.tile([128, 1], F32, tag="rec")
                nc.vector.reciprocal(rec, acc[:, D : D + 1])
                o_f = outp.tile([128, D], F32, tag="of")
                nc.scalar.activation(
                    o_f, acc[:, :D], mybir.ActivationFunctionType.Copy, scale=rec
                )
                amax = outp.tile([128, 1], F32, tag="amax")
                nc.vector.tensor_reduce(
                    amax, o_f, axis=mybir.AxisListType.X,
                    op=mybir.AluOpType.max, apply_absolute_value=True,
                )
                rinv = outp.tile([128, 1], F32, tag="rinv")
                nc.vector.reciprocal(rinv, amax)
                r127 = outp.tile([128, 1], F32, tag="r127")
                nc.scalar.activation(
                    r127, rinv, mybir.ActivationFunctionType.Copy, scale=127.0
                )
                o8 = outp.tile([128, D], I8, tag="o8")
                nc.scalar.activation(
                    o8, o_f, mybir.ActivationFunctionType.Copy, scale=r127
                )
                amax16 = outp.tile([128, 1], BF16, tag="amax16")
                nc.vector.tensor_copy(amax16, amax)
                nc.default_dma_engine.dma_start(
                    out=o_d[h, i * 128 : (i + 1) * 128, :], in_=o8
                )
                nc.default_dma_engine.dma_start(out=osc_d[h, i, :], in_=amax16)
    nc.compile()
    return nc


_NC_CACHE = None
_BUFS = None
_NEFF_MEMO = {}


def _install_neff_memo():
    """Content-keyed memo around the bass2jax neuronx_cc hook.

    run_bass_via_pjrt builds a fresh jax.jit per call, so XLA re-invokes the
    neuronx_cc hook (walrus BIR->NEFF compile, ~0.26s) on every call even
    though the BIR is identical. Cache the compiled NEFF by content hash;
    the kernel itself still executes on hardware every call.
    """
    import hashlib

    from concourse import bass2jax as _b2j

    inner = _b2j.neuronx_cc_hook
    if getattr(inner, "_neff_memo", False):
        return

    def memoized(code, code_format, platform_version, file_prefix):
        key_code = bytes(code)
        if bytes(code_format) == b"hlo":
            # The serialized module embeds a per-jit module id and the
            # caller's source location (stack_frame_index) — volatile
            # metadata that must not break the compile cache key.
            try:
                import libneuronxla.proto.hlo_pb2 as _hpb

                p = _hpb.HloModuleProto.FromString(key_code)
                p.ClearField("id")
                p.ClearField("stack_frame_index")
                key_code = p.SerializeToString()
            except Exception:
                pass
        key = hashlib.sha256(
            key_code + b"\x00" + bytes(code_format) + b"\x00"
            + str(platform_version).encode()
        ).digest()
        hit = _NEFF_MEMO.get(key)
        if hit is None:
            hit = inner(code, code_format, platform_version, file_prefix)
            _NEFF_MEMO[key] = hit
        return hit

    memoized._neff_memo = True
    _b2j.neuronx_cc_hook = memoized


def _get_bufs():
    global _BUFS
    if _BUFS is None:
        BH = B * H
        _BUFS = {
            "qv8": np.empty((BH, 2, S, D), np.int8),
            "k8": np.empty((BH, D, S), np.int8),
            "sc": np.empty((BH, 128, 3 * NQ), np.float32),
            "tmp": np.empty((S, D), np.float32),
            "tmpk": np.empty((D, S), np.float32),
        }
    return _BUFS


def _quant8(qf, kf, vf):
    """Blocked per-head int8 quantization into persistent buffers.

    rint(x * 127/absmax) is guaranteed within [-127, 127], so no clip pass.
    """
    bufs = _get_bufs()
    qv8, k8, sc = bufs["qv8"], bufs["k8"], bufs["sc"]
    tmp, tmpk = bufs["tmp"], bufs["tmpk"]
    for bh in range(B * H):
        x = qf[bh]
        qa = np.maximum(np.maximum(x.max(axis=-1), -x.min(axis=-1)), 1e-30)
        np.multiply(x, (127.0 / qa)[:, None], out=tmp)
        np.rint(tmp, out=tmp)
        np.copyto(qv8[bh, 0], tmp, casting="unsafe")
        sc[bh, :, 0:NQ] = (qa.reshape(NQ, 128) * (1.0 / 127.0)).T

        x = vf[bh]
        va = np.maximum(np.maximum(x.max(axis=-1), -x.min(axis=-1)), 1e-30)
        np.multiply(x, (127.0 / va)[:, None], out=tmp)
        np.rint(tmp, out=tmp)
        np.copyto(qv8[bh, 1], tmp, casting="unsafe")
        sc[bh, :, 2 * NQ : 3 * NQ] = (va.reshape(NQ, 128) * (1.0 / 127.0)).T

        x = kf[bh]
        ka = np.maximum(np.maximum(x.max(axis=0), -x.min(axis=0)), 1e-30)
        np.multiply(x, (127.0 / ka)[None, :], out=tmpk)
        np.rint(tmpk, out=tmpk)
        np.copyto(k8[bh], tmpk, casting="unsafe")
        sc[bh, :, NQ : 2 * NQ] = (
            ka.reshape(NQ, 128) * (SCALE / TANH_SCALE / 127.0)
        ).T
    return qv8, k8, sc


def kernel(q: np.ndarray, k: np.ndarray, v: np.ndarray) -> np.ndarray:
    global _NC_CACHE
    if _NC_CACHE is None:
        _install_neff_memo()
        _NC_CACHE = _build_nc()
    nc = _NC_CACHE

    q = np.asarray(q)
    k = np.asarray(k)
    v = np.asarray(v)
    qf = np.ascontiguousarray(q.reshape(B * H, S, D).astype(np.float32, copy=False))
    kf = np.ascontiguousarray(k.reshape(B * H, D, S).astype(np.float32, copy=False))
    vf = np.ascontiguousarray(v.reshape(B * H, S, D).astype(np.float32, copy=False))
    qv8, k8, sc = _quant8(qf, kf, vf)

    in_maps = []
    for c in range(N_CORES):
        sl = slice(c * HPC, (c + 1) * HPC)
        in_maps.append({"qv": qv8[sl], "k": k8[sl], "sc": sc[sl]})

    res = run_bass_kernel_spmd(nc, in_maps, core_ids=list(range(N_CORES)))
    out = np.empty((B * H, S, D), np.float32)
    for c in range(N_CORES):
        o8 = np.asarray(res.results[c]["o"]).reshape(HPC, S, D)
        osc = np.asarray(res.results[c]["osc"]).astype(np.float32).reshape(
            HPC, S, 1
        )
        np.multiply(o8, osc * (1.0 / 127.0), out=out[c * HPC : (c + 1) * HPC])
    return out.reshape(B, H, S, D)
